# revision 57
# baseline (speedup 1.0000x reference)
"""EdgeConv encoder for Trainium2 (Bass/Tile), v3.

Math (one EdgeConv layer, PyG semantics, aggr='add' over dst):
  u[n]  = x[n] @ (A_i - A_j).T + ba     (node-level)   A_i|A_j = wa split
  v[n]  = x[n] @ A_j.T                  (node-level)
  t_e   = relu(u[dst_e] + v[src_e])     (edge-level)
  agg[n] = sum_{e: dst_e = n} t_e       (scatter-add)
  conv[n] = agg[n] @ wb2 + deg[n] * c0  (node-level; BN+linear folded)
  layer1: h = l2norm(relu(conv)); layer2: out = conv

Sharding: edges partitioned by dst across 8 cores (each core owns 49
128-node blocks); outputs disjoint.

v3 design vs v2:
  - v written to DRAM node-major (v_lo/v_hi split at node 32768 for the
    int16 gather index limit); v[src] fetched with HBM-source
    NON-transpose dma_gather -> vg[e, ch, c] EDGE-major.  Concurrent
    transposed SBUF-source gathers corrupt data on this runtime (any >=2
    in flight); HBM-source non-transpose gathers are race-free at full
    queue concurrency, so the gather pipeline runs 4 queues deep with no
    serialization hazard.
  - message built edge-major: msg[e,c] = sT_chunk^T @ u_blk (one-hot
    stationary, streamed from HBM) accumulated with identity @ vg (one
    wide matmul per sub-batch).  ACT relu psum->t (bf16).  NO per-chunk
    PE transpose (t is already edge-major for the scatter).
  - scatter: aggT[c,n] += t_chunk^T @ s_chunk; s one-hot built on-chip
    with one DVE is_equal per sub-batch (ir8 vs dst_col broadcast).
  - dst one-hot sT precomputed on host, streamed from HBM (no PE
    broadcast matmul, no PSUM-read is_equal on DVE).
  - gather groups load-balanced by snake-dealing the sorted-rank slots
    so every group has a similar chunk total (bounds the vg/sT tiles).
"""

import sys

sys.path.insert(0, "/opt/trn_rl_repo")

import numpy as np

from concourse import bacc, bass, mybir, tile

F32 = mybir.dt.float32
BF16 = mybir.dt.bfloat16
I16 = mybir.dt.int16
BF16_NP = mybir.dt.np(BF16)
AF = mybir.ActivationFunctionType

C = 128
SB = 8             # chunks per message sub-batch (psum [128, 8*128] f32)
GB = 4             # blocks per gather group
MAXCH = 32         # chunks per dma_gather call
SPLIT = 32768      # int16 index split point (nodes >= SPLIT use v_hi)


def build_layer(n_blocks_total: int, blocks_per_core: int,
                sched_lo: list[int], sched_hi: list[int],
                apply_norm: bool, node_grp: int = 8):
    NBT, BPC = n_blocks_total, blocks_per_core
    blocks, groups, TC = make_layout(sched_lo, sched_hi, BPC, GB)
    nc = bacc.Bacc("TRN2", num_swdge_queues=4)

    # ---- inputs ----
    xt = nc.declare_dram_parameter("xt", [C, NBT * C], BF16, isOutput=False)
    xt_own = nc.declare_dram_parameter("xt_own", [C, BPC * C], BF16, isOutput=False)
    wv_t = nc.declare_dram_parameter("wv_t", [C, C], BF16, isOutput=False)
    wu_t = nc.declare_dram_parameter("wu_t", [C, C], BF16, isOutput=False)
    ba4 = nc.declare_dram_parameter("ba4", [1, 4 * C], BF16, isOutput=False)
    wb2 = nc.declare_dram_parameter("wb2", [C, C], BF16, isOutput=False)
    c0 = nc.declare_dram_parameter("c0", [1, C], BF16, isOutput=False)
    ident = nc.declare_dram_parameter("ident", [C, C], BF16, isOutput=False)
    ones_row = nc.declare_dram_parameter("ones_row", [1, C], BF16, isOutput=False)
    deg = nc.declare_dram_parameter("deg", [1, BPC * C], BF16, isOutput=False)
    src16 = nc.declare_dram_parameter("src16", [128, TC * 8], I16, isOutput=False)
    sT_hbm = nc.declare_dram_parameter("sT_hbm", [128, TC * C], BF16, isOutput=False)
    s_hbm = nc.declare_dram_parameter("s_hbm", [128, TC * C], BF16, isOutput=False)
    out_nm = nc.declare_dram_parameter("out_nm", [BPC * C, C], F32, isOutput=True)

    # ---- DRAM scratch: the v table, node-major rows ----
    v_lo = nc.dram_tensor("v_lo_scratch", [SPLIT, C], BF16)
    v_hi = nc.dram_tensor("v_hi_scratch", [NBT * C - SPLIT, C], BF16)

    max_gchunks = max((nlo + nhi for _, nlo, nhi, _ in groups), default=1)

    with tile.TileContext(nc) as tc:
        with (
            tc.tile_pool(name="persist", bufs=1) as persist,
            tc.tile_pool(name="nodeio", bufs=2) as nodeio,
            tc.tile_pool(name="vout", bufs=2) as vout,
            tc.tile_pool(name="edgeio", bufs=3) as edgeio,
            tc.tile_pool(name="tbuf", bufs=4) as tbuf,
            tc.tile_pool(name="outio", bufs=3) as outio,
            tc.tile_pool(name="widep", bufs=2, space="PSUM") as widep,
            tc.tile_pool(name="msgp", bufs=2, space="PSUM") as msgp,
            tc.tile_pool(name="aggp", bufs=1, space="PSUM") as aggp,
        ):
            # ---- persistent SBUF state ----
            srci_sb = persist.tile([128, TC * 8], I16, tag="srci")
            nc.sync.dma_start(out=srci_sb[:], in_=src16[:])
            wv_sb = persist.tile([C, C], BF16, tag="wv")
            nc.sync.dma_start(out=wv_sb[:], in_=wv_t[:])
            wu_sb = persist.tile([C, C], BF16, tag="wu")
            nc.sync.dma_start(out=wu_sb[:], in_=wu_t[:])
            ba4_sb = persist.tile([1, 4 * C], BF16, tag="ba4")
            nc.sync.dma_start(out=ba4_sb[:], in_=ba4[:])
            wb2_sb = persist.tile([C, C], BF16, tag="wb2")
            nc.sync.dma_start(out=wb2_sb[:], in_=wb2[:])
            c0_sb = persist.tile([1, C], BF16, tag="c0")
            nc.sync.dma_start(out=c0_sb[:], in_=c0[:])
            id_sb = persist.tile([C, C], BF16, tag="id")
            nc.sync.dma_start(out=id_sb[:], in_=ident[:])
            onesr_sb = persist.tile([1, C], BF16, tag="onesr")
            nc.sync.dma_start(out=onesr_sb[:], in_=ones_row[:])
            deg_sb = persist.tile([1, BPC * C], BF16, tag="deg")
            nc.sync.dma_start(out=deg_sb[:], in_=deg[:])
            u_sb = persist.tile([128, BPC * C], BF16, tag="u")

            # ================= node phase =================
            # PE warm-up: ~3.4us of dense 512-wide matmuls with a constant
            # stationary un-throttles the HAM clock gate before the v
            # sweep.  The sink is DMA'd into v_hi rows of PAD nodes
            # (>= 50000, never gathered) so nothing dead-code-eliminates.
            warm_sink = persist.tile([128, C], BF16, tag="warmsink")
            for w in range(8):
                wps = widep.tile([C, 4 * C], F32, tag="wide")
                nc.tensor.matmul(wps[:], lhsT=onesr_sb[:], rhs=ba4_sb[:],
                                 start=True, stop=True)
                if w == 7:
                    nc.vector.tensor_copy(out=warm_sink[:], in_=wps[:, :C])
            nc.scalar.dma_start(
                out=v_hi[NBT * C - SPLIT - 128: NBT * C - SPLIT, :]
                    .rearrange("(j p) c -> p j c", p=128),
                in_=warm_sink[:].rearrange("p (j c) -> p j c", c=C))

            # v[n] node-major -> v_lo/v_hi DRAM (lo first so lo gathers
            # can start as early as possible).  32-block staging slabs ->
            # one big DMA each (HWDGE issue cost amortized); slab DMAs on
            # the scalar HWDGE queue so they don't serialize behind the
            # sync queue's xt loads.
            VSLAB = 16
            for s0 in range(0, NBT, VSLAB):
                s1 = min(s0 + VSLAB, NBT)
                vstage = vout.tile([128, VSLAB * C], BF16, tag="vstage")
                for g0 in range(s0, s1, node_grp):
                    g1 = min(g0 + node_grp, s1)
                    xt_sb = nodeio.tile([C, node_grp * C], BF16, tag="xt")
                    nc.sync.dma_start(out=xt_sb[:, : (g1 - g0) * C],
                                      in_=xt[:, g0 * C: g1 * C])
                    for q0 in range(g0, g1, 4):
                        q1 = min(q0 + 4, g1)
                        nb = q1 - q0
                        vps = widep.tile([C, 4 * C], F32, tag="wide")
                        for b in range(q0, q1):
                            nc.tensor.matmul(
                                vps[:, (b - q0) * C: (b - q0 + 1) * C],
                                lhsT=xt_sb[:, (b - g0) * C: (b - g0 + 1) * C],
                                rhs=wv_sb[:], start=True, stop=True)
                        o0 = (q0 - s0) * C
                        if (q0 // 4) % 2 == 0:
                            nc.vector.tensor_copy(
                                out=vstage[:, o0: o0 + nb * C],
                                in_=vps[:, : nb * C])
                        else:
                            nc.scalar.activation(
                                out=vstage[:, o0: o0 + nb * C],
                                in_=vps[:, : nb * C], func=AF.Copy)
                r0, r1 = s0 * C, s1 * C
                dst = (v_lo[r0:r1, :] if r1 <= SPLIT
                       else v_hi[r0 - SPLIT: r1 - SPLIT, :])
                nc.scalar.dma_start(
                    out=dst.rearrange("(j p) c -> p j c", p=128),
                    in_=vstage[:, : (s1 - s0) * C]
                        .rearrange("p (j c) -> p j c", c=C))

            # u[n] for own (permuted) blocks -> u_sb node-major
            for g0 in range(0, BPC, 4):
                g1 = min(g0 + 4, BPC)
                nb = g1 - g0
                xo_sb = nodeio.tile([C, 4 * C], BF16, tag="xo")
                nc.sync.dma_start(out=xo_sb[:, : nb * C],
                                  in_=xt_own[:, g0 * C: g1 * C])
                ups = widep.tile([C, 4 * C], F32, tag="wide")
                # start=True clears has_written for the WHOLE bank, so only
                # the first matmul in the bank may carry it.
                for b in range(g0, g1):
                    nc.tensor.matmul(ups[:, (b - g0) * C: (b - g0 + 1) * C],
                                     lhsT=xo_sb[:, (b - g0) * C: (b - g0 + 1) * C],
                                     rhs=wu_sb[:], start=(b == g0), stop=False)
                nc.tensor.matmul(ups[:, : nb * C], lhsT=onesr_sb[:],
                                 rhs=ba4_sb[0:1, : nb * C],
                                 start=False, stop=True)
                nc.vector.tensor_copy(out=u_sb[:, g0 * C: g1 * C],
                                      in_=ups[:, : nb * C])

            # ================= edge phase =================
            grp_of_block = {}
            for gi_, (g_start, nlo_g, nhi_g, bs) in enumerate(groups):
                grp_of_block[bs[0]] = gi_

            vg_sb = None
            sTg_sb = None
            vg_base = 0
            gq = [0]

            # software pipeline: defer the scatter of the previous
            # sub-batch while the next sub-batch's message matmuls issue.
            pending = []

            def flush_pending():
                while pending:
                    pending.pop(0)()

            for b in range(BPC):
                lo0, nl, hi0, nh = blocks[b]
                nch = nl + nh
                if b in grp_of_block:
                    g_start, nlo_g, nhi_g, _ = groups[grp_of_block[b]]
                    vg_base = g_start
                    ng = nlo_g + nhi_g
                    if ng > 0:
                        vg_sb = edgeio.tile([128, max_gchunks, C], BF16,
                                            tag="vg")
                        for (cb, cn, src_dram) in ((0, nlo_g, v_lo),
                                                   (nlo_g, nhi_g, v_hi)):
                            for c0_ in range(0, cn, MAXCH):
                                cw = min(MAXCH, cn - c0_)
                                sl = g_start + cb + c0_
                                nc.gpsimd.dma_gather(
                                    out_ap=vg_sb[:, cb + c0_: cb + c0_ + cw, :],
                                    in_ap=src_dram[:],
                                    idxs_ap=srci_sb[:, sl * 8: (sl + cw) * 8],
                                    num_idxs=cw * C,
                                    num_idxs_reg=cw * C,
                                    elem_size=C,
                                    transpose=False,
                                    single_packet=False,
                                    queue_num=gq[0] % 4)
                                gq[0] += 1
                        sTg_sb = edgeio.tile([128, max_gchunks * C], BF16,
                                             tag="sTg")
                        nc.scalar.dma_start(
                            out=sTg_sb[:, : ng * C],
                            in_=sT_hbm[:, g_start * C: (g_start + ng) * C])
                        sg_sb = edgeio.tile([128, max_gchunks * C], BF16,
                                            tag="sg")
                        nc.scalar.dma_start(
                            out=sg_sb[:, : ng * C],
                            in_=s_hbm[:, g_start * C: (g_start + ng) * C])

                if nch == 0:
                    flush_pending()
                    agg_sb = outio.tile([C, C], BF16, tag="aggsb")
                    nc.gpsimd.memset(agg_sb[:], 0.0)
                else:
                    aggT = aggp.tile([C, C], F32, tag="agg")
                    ch_done = [0]
                    for (r0, rn) in ((lo0, nl), (hi0, nh)):
                        for j0 in range(0, rn, SB):
                            sw = min(SB, rn - j0)
                            rel = r0 + j0 - vg_base
                            slot0 = r0 + j0
                            msg = msgp.tile([128, SB * C], F32, tag="msg")
                            # start=True clears has_written for the WHOLE
                            # 512-f32 bank: carry it only on the first
                            # matmul touching each bank (chunks 0 and 4).
                            for j in range(sw):
                                nc.tensor.matmul(
                                    msg[:, j * C: (j + 1) * C],
                                    lhsT=sTg_sb[:, (rel + j) * C:
                                                (rel + j + 1) * C],
                                    rhs=u_sb[:, b * C: (b + 1) * C],
                                    start=(j % 4 == 0), stop=False)
                            for h0 in range(0, sw, 4):
                                hw = min(4, sw - h0)
                                nc.tensor.matmul(
                                    msg[:, h0 * C: (h0 + hw) * C],
                                    lhsT=id_sb[:],
                                    rhs=vg_sb[:, rel + h0: rel + h0 + hw, :]
                                        .rearrange("p j c -> p (j c)"),
                                    start=False, stop=True)
                            t_sb = tbuf.tile([128, SB * C], BF16, tag="t")
                            nc.scalar.activation(out=t_sb[:, : sw * C],
                                                 in_=msg[:, : sw * C],
                                                 func=AF.Relu)

                            def finish(t_sb=t_sb, sg_sb=sg_sb, sw=sw,
                                       rel=rel, aggT=aggT, ch_done=ch_done,
                                       nch=nch):
                                for j in range(sw):
                                    nc.tensor.matmul(
                                        aggT[:],
                                        lhsT=t_sb[:, j * C: (j + 1) * C],
                                        rhs=sg_sb[:, (rel + j) * C:
                                                  (rel + j + 1) * C],
                                        start=(ch_done[0] == 0),
                                        stop=(ch_done[0] == nch - 1))
                                    ch_done[0] += 1

                            flush_pending()
                            pending.append(finish)
                    flush_pending()
                    agg_sb = outio.tile([C, C], BF16, tag="aggsb")
                    nc.vector.tensor_copy(out=agg_sb[:], in_=aggT[:])

                # conv: node-major [n, c]
                cps = aggp.tile([C, C], F32, tag="conv")
                nc.tensor.matmul(cps[:], lhsT=agg_sb[:], rhs=wb2_sb[:],
                                 start=True, stop=False)
                nc.tensor.matmul(cps[:], lhsT=deg_sb[0:1, b * C: (b + 1) * C],
                                 rhs=c0_sb[:], start=False, stop=True)

                o_sb = outio.tile([C, C], F32, tag="o")
                if apply_norm:
                    h_sb = outio.tile([C, C], F32, tag="h")
                    nc.scalar.activation(out=h_sb[:], in_=cps[:],
                                         func=AF.Relu)
                    sq_sb = outio.tile([C, C], BF16, tag="sq")
                    nrm = outio.tile([C, 4], F32, tag="nrm")
                    nc.scalar.activation(out=sq_sb[:], in_=h_sb[:],
                                         func=AF.Square,
                                         accum_out=nrm[:, 0:1])
                    nc.scalar.activation(out=nrm[:, 1:2], in_=nrm[:, 0:1],
                                         func=AF.Sqrt)
                    nc.vector.tensor_scalar(out=nrm[:, 2:3], in0=nrm[:, 1:2],
                                            scalar1=1e-12, scalar2=None,
                                            op0=mybir.AluOpType.max)
                    nc.vector.reciprocal(out=nrm[:, 3:4], in_=nrm[:, 2:3])
                    nc.scalar.activation(out=o_sb[:], in_=h_sb[:],
                                         func=AF.Copy, scale=nrm[:, 3:4])
                else:
                    nc.scalar.activation(out=o_sb[:], in_=cps[:],
                                         func=AF.Copy)
                nc.sync.dma_start(out=out_nm[b * C: (b + 1) * C, :],
                                  in_=o_sb[:])

    nc.compile()
    return nc


# ---------------- host-side data prep ----------------


def make_layout(sched_lo, sched_hi, bpc, gather_blocks=GB):
    """Group-major slot order: per gather group, all lo slots (block-major)
    then all hi slots. Returns per-block (lo_start, nlo, hi_start, nhi),
    group list (chunk_start, nlo_g, nhi_g, blocks)."""
    blocks = []
    groups = []
    pos = 0
    b = 0
    while b < bpc:
        bs = list(range(b, min(b + gather_blocks, bpc)))
        g_start = pos
        lo_starts = {}
        for bb in bs:
            lo_starts[bb] = pos
            pos += sched_lo[bb]
        nlo_g = pos - g_start
        hi_starts = {}
        for bb in bs:
            hi_starts[bb] = pos
            pos += sched_hi[bb]
        nhi_g = pos - g_start - nlo_g
        for bb in bs:
            blocks.append((lo_starts[bb], sched_lo[bb],
                           hi_starts[bb], sched_hi[bb]))
        groups.append((g_start, nlo_g, nhi_g, bs))
        b += gather_blocks
    return blocks, groups, pos


def snake_deal(bpc, gather_blocks=GB):
    """Position->rank map: deal sorted ranks round-robin into groups so
    each gather group's chunk total is balanced (rank 0 = busiest)."""
    ngroups = (bpc + gather_blocks - 1) // gather_blocks
    pos2rank = []
    for g in range(ngroups):
        for i in range(gather_blocks):
            r = i * ngroups + g
            if r < bpc:
                pos2rank.append(r)
    assert sorted(pos2rank) == list(range(bpc))
    return pos2rank


def prep_edges(src, dst, n_cores, bpc, gather_blocks=GB):
    """Partition edges by dst core/block, split each block's edges into
    lo (src < SPLIT) and hi chunks for int16 dma_gather indexing.

    Each core's blocks are permuted so slot positions hold matched-rank
    blocks across cores (sorted-rank matching), then slots are
    snake-dealt into gather groups for balanced group sizes."""
    npc = bpc * C
    order = np.argsort(dst, kind="stable")
    src_s, dst_s = src[order], dst[order]
    core_lists = []
    nlo = np.zeros((n_cores, bpc), np.int64)
    nhi = np.zeros((n_cores, bpc), np.int64)
    for k in range(n_cores):
        lo_ = np.searchsorted(dst_s, k * npc, side="left")
        hi_ = np.searchsorted(dst_s, (k + 1) * npc, side="left")
        s_k, d_k = src_s[lo_:hi_], dst_s[lo_:hi_] - k * npc
        blk = d_k // C
        per_blk = []
        for b in range(bpc):
            m = blk == b
            sb, db = s_k[m], d_k[m] - b * C
            so_ = np.argsort(sb, kind="stable")
            sb, db = sb[so_], db[so_]  # src-sorted: ascending gather addrs
            isl = sb < SPLIT
            per_blk.append(((sb[isl], db[isl]), (sb[~isl], db[~isl])))
            nlo[k, b] = isl.sum()
            nhi[k, b] = (~isl).sum()
        core_lists.append(per_blk)

    # sorted-rank matching by total block count, then snake-deal ranks to
    # positions for balanced gather groups
    ntot = nlo + nhi
    rank_perms = [np.argsort(-ntot[k], kind="stable") for k in range(n_cores)]
    pos2rank = snake_deal(bpc, gather_blocks)
    perms = [rank_perms[k][pos2rank] for k in range(n_cores)]
    nlo_p = np.stack([nlo[k][perms[k]] for k in range(n_cores)])
    nhi_p = np.stack([nhi[k][perms[k]] for k in range(n_cores)])
    chlo = np.ceil(nlo_p / C).astype(np.int64)
    chhi = np.ceil(nhi_p / C).astype(np.int64)
    sched_lo = [int(x) for x in chlo.max(axis=0)]
    sched_hi = [int(x) for x in chhi.max(axis=0)]
    blocks, groups, TC = make_layout(sched_lo, sched_hi, bpc, gather_blocks)

    per_core = []
    for k in range(n_cores):
        si16 = np.zeros((16, TC * 8), np.int16)
        db_ = np.full((TC, C), 200.0, np.float64)
        for b in range(bpc):
            gb = perms[k][b]
            (slo, sdlo), (shi, sdhi) = core_lists[k][gb]
            lo0, nl, hi0, nh = blocks[b]
            for (vals, dvals, base, nslots, off) in (
                    (slo, sdlo, lo0, nl, 0),
                    (shi, sdhi, hi0, nh, SPLIT)):
                n = len(vals)
                if nslots == 0:
                    continue
                idx = np.arange(n)
                ch = base + idx // C
                lane = idx % C
                iv = (vals - off).astype(np.int16)
                si16[lane % 16, ch * 8 + lane // 16] = iv
                db_[ch, lane] = dvals
        full = np.zeros((128, TC * 8), np.int16)
        for rr in range(8):
            full[rr * 16: (rr + 1) * 16] = si16
        # dst one-hots: sT[n, slot*C + e] (msg stationary) and
        # s[e, slot*C + n] (scatter rhs)
        sl_, ee_ = np.nonzero(db_ < 128)
        dv_ = db_[sl_, ee_].astype(np.int64)
        sT = np.zeros((128, TC, C), BF16_NP)
        sT[dv_, sl_, ee_] = 1
        s_ = np.zeros((128, TC, C), BF16_NP)
        s_[ee_, sl_, dv_] = 1
        per_core.append({
            "src16": full,
            "sT": np.ascontiguousarray(sT.reshape(128, TC * C)),
            "s": np.ascontiguousarray(s_.reshape(128, TC * C)),
            "perm": perms[k],
        })
    return sched_lo, sched_hi, per_core


def fold_weights(wa, ba_, g, be, rm, rv, wb, bb, bn_eps=1e-5):
    wa = wa.astype(np.float64)
    A_i, A_j = wa[:, :C], wa[:, C:]
    s = g.astype(np.float64) / np.sqrt(rv.astype(np.float64) + bn_eps)
    wb64 = wb.astype(np.float64)
    wu_t = (A_i - A_j).T
    wv_t = A_j.T
    wb2 = s[:, None] * wb64.T          # wb2[j, i] = s_j * wb[i, j]
    c0 = bb.astype(np.float64) + (be.astype(np.float64) - rm.astype(np.float64) * s) @ wb64.T
    return (wu_t.astype(BF16_NP), wv_t.astype(BF16_NP),
            ba_.astype(BF16_NP).reshape(1, C),
            wb2.astype(BF16_NP), c0.astype(BF16_NP).reshape(1, C))


# ======================================================================
# Full-problem kernel: 2-layer EdgeConv encoder, N=50000, E=600000, C=128
# ======================================================================

import os

N_NODES = 50000
N_EDGES = 600000
CORES = 8
BPC = 49                  # blocks per core
NBT = CORES * BPC         # 392 blocks total
NP = NBT * C              # padded node count 50176
BN_EPS = 1e-5

LAST = {}                 # timing/info stash for test harness


def _prep_all(x, edge_index):
    src = np.asarray(edge_index[0], np.int64).astype(np.int32)
    dst = np.asarray(edge_index[1], np.int64).astype(np.int32)
    sched_lo, sched_hi, per_core = prep_edges(src, dst, CORES, BPC)
    deg_full = np.bincount(dst, minlength=NP).astype(np.float64)
    x_pad = np.zeros((NP, C), np.float32)
    x_pad[:N_NODES] = x
    xt = np.ascontiguousarray(x_pad.T).astype(BF16_NP)
    return sched_lo, sched_hi, per_core, deg_full, xt


def _layer_inputs(xt_bf16, per_core, deg_full, wset):
    wu_t, wv_t, ba_f, wb2, c0 = wset
    ident = np.eye(C, dtype=np.float64).astype(BF16_NP)
    onesr = np.ones((1, C), dtype=BF16_NP)
    ba4 = np.tile(ba_f, (1, 4)).astype(BF16_NP)
    in_maps = []
    for k in range(CORES):
        npc = BPC * C
        perm = per_core[k]["perm"]
        xo = xt_bf16[:, k * npc: (k + 1) * npc].reshape(C, BPC, C)
        dg = deg_full[k * npc: (k + 1) * npc].reshape(BPC, C)
        in_maps.append({
            "xt": xt_bf16,
            "xt_own": np.ascontiguousarray(
                xo[:, perm, :].reshape(C, npc)),
            "wv_t": wv_t, "wu_t": wu_t, "ba4": ba4, "wb2": wb2, "c0": c0,
            "ident": ident, "ones_row": onesr,
            "deg": np.ascontiguousarray(
                dg[perm].reshape(1, npc).astype(BF16_NP)),
            "src16": per_core[k]["src16"],
            "sT_hbm": per_core[k]["sT"],
            "s_hbm": per_core[k]["s"],
        })
    return in_maps


_NTFF_HOOK = None


def _get_ntff_hook():
    global _NTFF_HOOK
    if _NTFF_HOOK is None:
        sys.path.insert(0, "/root/.axon_site")
        from trn_agent_boot.trn_boot import _ntff_profile_via_ctypes
        _NTFF_HOOK = _ntff_profile_via_ctypes("/opt/axon/libaxon_pjrt.so")
    return _NTFF_HOOK


def _run(nc, in_maps):
    import tempfile
    from concourse import bass2jax
    trace = bool(int(os.environ.get("EDGECONV_TRACE", "0")))
    hook = _get_ntff_hook() if trace else None
    if hook is None:
        results = bass2jax.run_bass_via_pjrt(nc, in_maps, n_cores=CORES)
        LAST.setdefault("exec_ns", []).append(None)
        return results
    neff_dir = tempfile.mkdtemp(prefix="edgeconv_ntff_")
    with hook(neff_dir, [0]):
        results = bass2jax.run_bass_via_pjrt(nc, in_maps, n_cores=CORES)
    exec_ns = None
    try:
        import glob as _glob
        import gauge.profiler
        from concourse._compat import FishPath
        if _glob.glob(os.path.join(neff_dir, "*_body*.ntff")):
            profile = gauge.profiler.Profile(
                profile_path=FishPath(neff_dir), kernel_dev_mode=True,
                profile_on_exit=False, bass_kernel=nc.m,
                offline_processing=True, fname="*_body*")
            pr = profile.to_perfetto(model_index=(0,))
            if pr:
                exec_ns = pr[0].exec_time_ns
                LAST.setdefault("trace_paths", []).append(pr[0].trace_path)
    except Exception as e:  # profiling must never break the kernel
        LAST.setdefault("trace_errors", []).append(repr(e))
    LAST.setdefault("neff_dirs", []).append(neff_dir)
    LAST.setdefault("exec_ns", []).append(exec_ns)
    return results


def kernel(**inputs):
    x = np.asarray(inputs["x"], np.float32)
    edge_index = np.asarray(inputs["edge_index"])
    sched_lo, sched_hi, per_core, deg_full, xt = _prep_all(x, edge_index)

    w1 = fold_weights(np.asarray(inputs["w1a"]), np.asarray(inputs["b1a"]),
                      np.asarray(inputs["g1"]), np.asarray(inputs["be1"]),
                      np.asarray(inputs["rm1"]), np.asarray(inputs["rv1"]),
                      np.asarray(inputs["w1b"]), np.asarray(inputs["b1b"]),
                      BN_EPS)
    w2 = fold_weights(np.asarray(inputs["w2a"]), np.asarray(inputs["b2a"]),
                      np.asarray(inputs["g2"]), np.asarray(inputs["be2"]),
                      np.asarray(inputs["rm2"]), np.asarray(inputs["rv2"]),
                      np.asarray(inputs["w2b"]), np.asarray(inputs["b2b"]),
                      BN_EPS)

    def unperm(rs):
        out = np.empty((NP, C), np.float32)
        for k, r in enumerate(rs):
            o = np.asarray(r["out_nm"], np.float32).reshape(BPC, C, C)
            out.reshape(NBT, C, C)[k * BPC + per_core[k]["perm"]] = o
        return out

    nc1 = build_layer(NBT, BPC, sched_lo, sched_hi, apply_norm=True)
    r1 = _run(nc1, _layer_inputs(xt, per_core, deg_full, w1))
    h = unperm(r1)                                    # [NP, C] node-major
    xt2 = np.ascontiguousarray(h.T).astype(BF16_NP)   # [C, NP] feature-major

    nc2 = build_layer(NBT, BPC, sched_lo, sched_hi, apply_norm=False)
    r2 = _run(nc2, _layer_inputs(xt2, per_core, deg_full, w2))
    out = unperm(r2)
    return np.ascontiguousarray(out[:N_NODES]).astype(np.float32)
